# revision 8
# baseline (speedup 1.0000x reference)
"""Bass/Trainium2 kernel for nn_DiffAllocator (64x7 Sinkhorn, 200 iterations).

The reference runs 200 log-domain Sinkhorn iterations; iteration 1 (the only
one needing exact max-stabilized LSE) runs on host as input prep, iterations
2..200 run on device in a multiplicative basis anchored at stabilizers
(pa, psi):

    A2  = exp(K + pa (+)rows psi (+)cols)              # [64,7]
    A1s = -sigma * exp(K + (pa-la) (+) (psi+lb))       # [64,7], sigma<0

Inner loop per iteration (critical path ~435 ns):
    PE:  CB = matmul(lhsT=u bcast [64,64] (stride-0), rhs=A2)   # CB[l,b]=c[b]
    DVE: RMR custom op: r[l] = sum_b A1s[l,b] * ~recip(CB[l,b]) # one instr
    DVE: u' = 1/r                                                # [64,1], free

RMR is a custom DVE program (registered at runtime):
    n = bitwise_not(c); p = c*n in [-4.5,-4];
    out = (n*(GAMMA + p*(BETA+p))) * Src1 ; accum = rowsum(out)
i.e. an 8-stage approximate reciprocal (monic quadratic minimax of 1/p,
rel err 5.2e-5; overall scale sigma folded into the static A1s) fused with
the multiply by A1 and the row reduction.  Only one PE->DVE round trip per
iteration; the second contraction is the row-sum inside the DVE op.

Every W=16 iterations the stabilizers absorb the accumulated (u, c) (fold):
ln u via the gpsimd bitcast-approx ln, ln c via an ACT copy of one CB row +
the same approx; the new basis matrices A2', A1s' and a one-iteration switch
matrix A2sw = exp(K + pa_old (+) psi_new) are regenerated on Pool+ACT fully
off the critical path, and the basis switches DELAY=8 iterations later.
Final P = diag(u) A2 diag(b*q), q = 1/c from the last CB row; assembled on
Pool in [64,7] layout (no transposes anywhere on device).
"""

import numpy as np

L, B = 64, 7
EPS = 0.02
ITERS = 200
W = 16      # fold window
DELAY = 8   # iterations between fold snapshot and basis switch (must be < W)

# monic quadratic minimax of 1/p over p in [-4.5,-4] (Remez, rel err 5.2e-5)
SIG = -0.013060559
BETA = 12.75
GAMMA = 54.140624
LN2 = float(np.log(2.0))
GLC1, GLC2 = LN2 / (2.0 ** 23), -127.0 * LN2   # gpsimd bitcast-ln constants

_CACHE = {}


def _register_rmr():
    """Register the RECIP_MUL_REDUCE_A custom DVE op (idempotent)."""
    import concourse.dve_ops as dve_ops
    from concourse.dve_ops import DveOp
    from concourse.dve_spec import AluOp, Bin, Spec, Src0, Src1, Zero, C0, C1
    from operator import add

    if "RECIP_MUL_REDUCE_A" in dve_ops.CUSTOM_DVE_SPECS:
        return next(o for o in dve_ops.OPS if o.name == "RECIP_MUL_REDUCE_A")

    n = Bin(AluOp.BITWISE_NOT, Src0, Src0)
    p = Src0 * n
    v = C1 + p * (C0 + p)               # monic quadratic, 3 ops
    body = (n * v) * Src1               # 7 ops total; accum takes stage 8

    def _ref(in0, in1, c0, c1, c2):
        x = np.ascontiguousarray(in0.astype(np.float32))
        nn = (~x.view(np.int32)).view(np.float32)
        pp = (x * nn).astype(np.float32)
        u1 = (np.float32(c0) + pp).astype(np.float32)
        u2 = (pp * u1).astype(np.float32)
        vv = (np.float32(c1) + u2).astype(np.float32)
        y = (nn * vv).astype(np.float32)
        b = (y * in1.astype(np.float32)).astype(np.float32)
        return b, b.reshape(b.shape[0], -1).sum(axis=-1, keepdims=True)

    spec = Spec(body=body, accum=add, accum_init=Zero, reference=_ref)

    dve_ops._SUB_OPCODE_FOR_NAME["RECIP_MUL_REDUCE_A"] = (
        dve_ops._CUSTOM_DVE_ROW_BASE + len(dve_ops.OPS))
    assert dve_ops._SUB_OPCODE_FOR_NAME["RECIP_MUL_REDUCE_A"] < 0x20

    def make(shas):
        return DveOp("RECIP_MUL_REDUCE_A", spec, subdim=False, uops_sha=shas)

    op = make({"v3": "?", "v4": "?"})
    for ver in ("v3", "v4"):
        try:
            op.compile(ver)
        except ValueError as e:
            import re
            m = re.search(r'uops_sha\["(v\d)"\]="([0-9a-f]+)"', str(e))
            assert m, str(e)
            shas = dict(op.uops_sha)
            shas[m.group(1)] = m.group(2)
            op = make(shas)
    op.compile("v3"); op.compile("v4")

    dve_ops.OPS.append(op)
    dve_ops.CUSTOM_DVE_SPECS[op.name] = op.spec
    return op


def _build_nc():
    import concourse.bacc as bacc
    import concourse.tile as tile
    import concourse.bass as bass
    import concourse.mybir as mybir

    rmr_op = _register_rmr()

    f32 = mybir.dt.float32
    AF = mybir.ActivationFunctionType
    OP = mybir.AluOpType
    MS = bass.MemorySpace

    nc = bacc.Bacc("TRN2", target_bir_lowering=False, debug=False)

    # packed inputs:
    #  WA [64, 31] = [A2_0 | A1s_0 | K | CB0 | la | pa0 | mla]
    #  RW [1, 21]  = [lb_row | b_row | psi0_row]
    d_WA = nc.dram_tensor("WA_in", [L, 31], f32, kind="ExternalInput").ap()
    d_RW = nc.dram_tensor("RW_in", [1, 21], f32, kind="ExternalInput").ap()
    d_P = nc.dram_tensor("P_out", [L, B], f32, kind="ExternalOutput").ap()

    with tile.TileContext(nc) as tc:
        with (
            tc.tile_pool(name="sb", bufs=1) as sb,
            tc.tile_pool(name="ps", bufs=2, space=MS.PSUM) as ps,
        ):
            def t(shape, tag):
                return sb.tile(shape, f32, tag=tag, name=tag)

            WA = t([L, 31], "WA")
            RW = t([1, 21], "RW")
            A2 = [WA[:, 0:7], t([L, B], "A2_1")]
            A1s = [WA[:, 7:14], t([L, B], "A1s_1")]
            K = WA[:, 14:21]
            CB0 = WA[:, 21:28]
            la = WA[:, 28:29]
            pa = [WA[:, 29:30], t([L, 1], "pa_1")]
            mla = WA[:, 30:31]                      # -la + ln|sigma|
            lb_row = RW[:, 0:7]
            b_row = RW[:, 7:14]
            psi = [RW[:, 14:21], t([1, B], "psi_1")]

            u = [t([L, 1], f"u{i}") for i in range(4)]
            r = [t([L, 1], "r0"), t([L, 1], "r1")]
            scr = [t([L, B], "scr0"), t([L, B], "scr1")]
            A2sw = t([L, B], "A2sw")
            paA1 = t([L, 1], "paA1")
            yf = t([L, 1], "yf")
            lnu = t([L, 1], "lnu")
            crow = t([1, B], "crow")
            yfr = t([1, B], "yfr")
            lncr = t([1, B], "lncr")
            psiT = t([1, B], "psiT")
            psiB = t([1, B], "psiB")
            PSBa = t([L, B], "PSBa")
            PSBb = t([L, B], "PSBb")
            T1a = t([L, B], "T1a")
            T1b = t([L, B], "T1b")
            qrow = t([1, B], "qrow")
            bq = t([1, B], "bq")
            BQ = t([L, B], "BQ")
            Pu = t([L, B], "Pu")
            Pf = t([L, B], "Pf")

            # ---- load inputs ----
            nc.sync.dma_start(out=WA[:], in_=d_WA)
            nc.gpsimd.dma_start(out=RW[:], in_=d_RW)

            # dummy Exp pulls the ACT table load into the head
            nc.scalar.activation(crow[:], lb_row, AF.Exp)

            def gps_ln(out_ap, yf_ap, x_ap):
                nc.gpsimd.tensor_copy(yf_ap, x_ap.bitcast(mybir.dt.uint32))
                nc.gpsimd.tensor_scalar(out=out_ap, in0=yf_ap, scalar1=GLC1,
                                        scalar2=GLC2, op0=OP.mult, op1=OP.add)

            epoch = 0
            fp = 0              # fold parity for pa/psi ping-pong
            switch_at = None
            deferred = {}
            prev_cb = None
            last_cb = None

            for it in range(2, ITERS + 1):
                par = it % 2
                par4 = it % 4
                if switch_at == it:
                    epoch = 1 - epoch
                    switch_at = None
                # RMR + recip
                in0 = CB0 if it == 2 else prev_cb[:]
                nc.vector._custom_dve(
                    rmr_op, out=scr[par][:], in0=in0, in1=A1s[epoch][:],
                    s0=BETA, s1=GAMMA, accum_out=r[par][:],
                )
                nc.vector.reciprocal(u[par4][:], r[par][:])
                # PE: CB_it (also emitted for it=200: needed for final q)
                cb = ps.tile([L, B], f32, tag="cb", bufs=3)
                ub = u[par4][:, 0:1].broadcast_to((L, L))
                rhs = A2sw if (switch_at == it + 1) else A2[epoch]
                nc.tensor.matmul(cb[:], ub, rhs[:], start=True, stop=True)
                prev_cb, last_cb = cb, cb

                # fold: snapshot u_it (Pool) and c_it (ACT copy of CB row).
                # c comes from a duplicate fold-only matmul so the loop's cb
                # tile keeps a single (DVE) reader - a second reader makes
                # Tile block the DVE sequencer on an EventSemaphore.
                if it % W == 0 and it + DELAY < ITERS and it >= W:
                    ne = 1 - epoch
                    nf = 1 - fp
                    cbf = ps.tile([L, B], f32, tag="cbf", bufs=1)
                    nc.tensor.matmul(cbf[:], ub, rhs[:], start=True, stop=True)
                    # ln u = -ln r (exact device recip), negated gpsimd-ln
                    nc.gpsimd.tensor_copy(yf[:], r[par][:].bitcast(mybir.dt.uint32))
                    nc.gpsimd.tensor_scalar(out=lnu[:], in0=yf[:], scalar1=-GLC1,
                                            scalar2=-GLC2, op0=OP.mult, op1=OP.add)
                    nc.gpsimd.tensor_scalar(out=pa[nf][:], in0=lnu[:],
                                            scalar1=pa[fp][:], scalar2=la,
                                            op0=OP.add, op1=OP.add)
                    nc.gpsimd.tensor_scalar(out=paA1[:], in0=pa[nf][:],
                                            scalar1=mla, scalar2=None,
                                            op0=OP.add)
                    nc.scalar.activation(crow[:], cbf[0:1, :], AF.Copy)

                    def stage_b(nf=nf, fp=fp):
                        gps_ln(lncr[:], yfr[:], crow[:])
                        nc.gpsimd.tensor_tensor(out=psiT[:], in0=psi[fp][:],
                                                in1=lb_row, op=OP.add)
                        nc.gpsimd.tensor_tensor(out=psi[nf][:], in0=psiT[:],
                                                in1=lncr[:], op=OP.subtract)
                        nc.gpsimd.tensor_tensor(out=psiB[:], in0=psi[nf][:],
                                                in1=lb_row, op=OP.add)

                    def stage_c(nf=nf):
                        nc.gpsimd.partition_broadcast(PSBa[:], psi[nf][:])
                        nc.gpsimd.tensor_tensor(out=T1a[:], in0=K, in1=PSBa[:],
                                                op=OP.add)
                        nc.gpsimd.partition_broadcast(PSBb[:], psiB[:])
                        nc.gpsimd.tensor_tensor(out=T1b[:], in0=K, in1=PSBb[:],
                                                op=OP.add)

                    def stage_d(ne=ne, nf=nf, fp=fp):
                        nc.scalar.activation(A2[ne][:], T1a[:], AF.Exp,
                                             bias=pa[nf][:])
                        nc.scalar.activation(A2sw[:], T1a[:], AF.Exp,
                                             bias=pa[fp][:])
                        nc.scalar.activation(A1s[ne][:], T1b[:], AF.Exp,
                                             bias=paA1[:])

                    def stage_e(ne=ne):
                        nc.gpsimd.tensor_scalar(out=A1s[ne][:], in0=A1s[ne][:],
                                                scalar1=-1.0, scalar2=None,
                                                op0=OP.mult)

                    deferred.setdefault(it + 2, []).append(stage_b)
                    deferred.setdefault(it + 3, []).append(stage_c)
                    deferred.setdefault(it + 4, []).append(stage_d)
                    deferred.setdefault(it + 5, []).append(stage_e)
                    switch_at = it + DELAY
                    fp = nf

                for fn in deferred.pop(it, []):
                    fn()

            # ---- final: P = diag(u_200) A2 diag(b * (1/c_200)) ----
            fpar = ITERS % 4
            nc.gpsimd.tensor_scalar(out=Pu[:], in0=A2[epoch][:],
                                    scalar1=u[fpar][:], scalar2=None,
                                    op0=OP.mult)
            nc.vector.reciprocal(qrow[:], last_cb[0:1, :])
            nc.gpsimd.tensor_tensor(out=bq[:], in0=qrow[:], in1=b_row,
                                    op=OP.mult)
            nc.gpsimd.partition_broadcast(BQ[:], bq[:])
            nc.gpsimd.tensor_tensor(out=Pf[:], in0=Pu[:], in1=BQ[:],
                                    op=OP.mult)
            nc.sync.dma_start(out=d_P, in_=Pf[:])

    nc.compile()
    return nc


def _host_inputs(theta, phi, n, sens, err):
    f32 = np.float32
    theta = np.asarray(theta, f32); phi = np.asarray(phi, f32)
    n = np.asarray(n, f32); sens = np.asarray(sens, f32)
    err = np.asarray(err, f32)
    a = (n / n.sum()).astype(f32)
    e = np.exp((phi - phi.max()).astype(f32)); b = (e / e.sum()).astype(f32)
    C = ((n * sens)[:, None] * err[None, :]).astype(f32)
    K = ((theta - C) * f32(1.0 / EPS)).astype(f32)
    la = np.log(a).astype(f32)
    lb = np.log(b).astype(f32)
    lnsig = f32(np.log(-SIG))

    def lse(x, axis):
        m = x.max(axis=axis, keepdims=True)
        return (m + np.log(np.exp(x - m).sum(axis=axis, keepdims=True))
                ).squeeze(axis).astype(f32)

    def ftz(x):
        x = np.asarray(x, f32).copy()
        x[np.abs(x) < 1.17549435e-38] = 0.0
        return x

    # iteration 1 (log domain, max-stabilized LSE) on host
    f1 = (la - lse(K, 1)).astype(f32)
    g1 = (lb - lse(K + f1[:, None], 0)).astype(f32)
    pa0 = (f1 + la).astype(f32)
    psi0 = g1.astype(f32)
    A2_0 = ftz(np.exp((K + pa0[:, None] + psi0[None, :]).astype(f32)))
    A1s_0 = -ftz(np.exp((K + (pa0 - la + lnsig)[:, None]
                         + (psi0 + lb)[None, :]).astype(f32)))
    CB0 = np.broadcast_to(b[None, :], (L, B)).astype(f32)
    mla = (-la + lnsig).astype(f32)

    WA = np.concatenate(
        [A2_0, A1s_0, K, CB0,
         np.stack([la, pa0, mla], axis=1)], axis=1).astype(f32)
    RW = np.concatenate([lb, b, psi0])[None, :].astype(f32)
    return {"WA_in": np.ascontiguousarray(WA),
            "RW_in": np.ascontiguousarray(RW)}


def kernel(theta, phi, n, sens, err):
    if "nc" not in _CACHE:
        _CACHE["nc"] = _build_nc()
    nc = _CACHE["nc"]
    in_map = _host_inputs(theta, phi, n, sens, err)
    from concourse import bass_utils
    res = bass_utils.run_bass_kernel_spmd(nc, [in_map], [0])
    return np.asarray(res.results[0]["P_out"], dtype=np.float32)


# revision 13
# speedup vs baseline: 1.0088x; 1.0088x over previous
"""Bass/Trainium2 kernel for nn_DiffAllocator (64x7 Sinkhorn, 200 iterations).

The reference runs 200 log-domain Sinkhorn iterations; iteration 1 (the only
one needing exact max-stabilized LSE) runs on host as input prep, iterations
2..200 run on device in a multiplicative basis anchored at stabilizers
(pa, psi):

    A2  = exp(K + pa (+)rows psi (+)cols)              # [64,7]
    A1s = -sigma * exp(K + (pa-la) (+) (psi+lb))       # [64,7], sigma<0

Inner loop per iteration (critical path ~435 ns):
    PE:  CB = matmul(lhsT=u bcast [64,64] (stride-0), rhs=A2)   # CB[l,b]=c[b]
    DVE: RMR custom op: r[l] = sum_b A1s[l,b] * ~recip(CB[l,b]) # one instr
    DVE: u' = 1/r                                                # [64,1], free

RMR is a custom DVE program (registered at runtime):
    n = bitwise_not(c); p = c*n in [-4.5,-4];
    out = (n*(GAMMA + p*(BETA+p))) * Src1 ; accum = rowsum(out)
i.e. an 8-stage approximate reciprocal (monic quadratic minimax of 1/p,
rel err 5.2e-5; overall scale sigma folded into the static A1s) fused with
the multiply by A1 and the row reduction.  Only one PE->DVE round trip per
iteration; the second contraction is the row-sum inside the DVE op.

Every W=16 iterations the stabilizers absorb the accumulated (u, c) (fold):
ln u via the gpsimd bitcast-approx ln, ln c via an ACT copy of one CB row +
the same approx; the new basis matrices A2', A1s' and a one-iteration switch
matrix A2sw = exp(K + pa_old (+) psi_new) are regenerated on Pool+ACT fully
off the critical path, and the basis switches DELAY=8 iterations later.
Final P = diag(u) A2 diag(b*q), q = 1/c from the last CB row; assembled on
Pool in [64,7] layout (no transposes anywhere on device).
"""

import numpy as np

L, B = 64, 7
EPS = 0.02
ITERS = 200
W = 16      # base fold cadence (drift-limited early)
DELAY = 8   # iterations between fold snapshot and basis switch
# drift slows as the trajectory converges: fewer folds late (validated in the
# numpy device-model against the reference)
FOLD_ITS = frozenset({16, 32, 48, 64, 96, 128, 160})

# monic quadratic minimax of 1/p over p in [-4.5,-4] (Remez, rel err 5.2e-5)
SIG = -0.013060559
BETA = 12.75
GAMMA = 54.140624
LN2 = float(np.log(2.0))
GLC1, GLC2 = LN2 / (2.0 ** 23), -127.0 * LN2   # gpsimd bitcast-ln constants

_CACHE = {}


def _register_rmr():
    """Register the RECIP_MUL_REDUCE_A custom DVE op (idempotent)."""
    import concourse.dve_ops as dve_ops
    from concourse.dve_ops import DveOp
    from concourse.dve_spec import AluOp, Bin, Spec, Src0, Src1, Zero, C0, C1
    from operator import add

    if "RECIP_MUL_REDUCE_A" in dve_ops.CUSTOM_DVE_SPECS:
        return next(o for o in dve_ops.OPS if o.name == "RECIP_MUL_REDUCE_A")

    n = Bin(AluOp.BITWISE_NOT, Src0, Src0)
    p = Src0 * n
    v = C1 + p * (C0 + p)               # monic quadratic, 3 ops
    body = (n * v) * Src1               # 7 ops total; accum takes stage 8

    def _ref(in0, in1, c0, c1, c2):
        x = np.ascontiguousarray(in0.astype(np.float32))
        nn = (~x.view(np.int32)).view(np.float32)
        pp = (x * nn).astype(np.float32)
        u1 = (np.float32(c0) + pp).astype(np.float32)
        u2 = (pp * u1).astype(np.float32)
        vv = (np.float32(c1) + u2).astype(np.float32)
        y = (nn * vv).astype(np.float32)
        b = (y * in1.astype(np.float32)).astype(np.float32)
        return b, b.reshape(b.shape[0], -1).sum(axis=-1, keepdims=True)

    spec = Spec(body=body, accum=add, accum_init=Zero, reference=_ref)

    dve_ops._SUB_OPCODE_FOR_NAME["RECIP_MUL_REDUCE_A"] = (
        dve_ops._CUSTOM_DVE_ROW_BASE + len(dve_ops.OPS))
    assert dve_ops._SUB_OPCODE_FOR_NAME["RECIP_MUL_REDUCE_A"] < 0x20

    def make(shas):
        return DveOp("RECIP_MUL_REDUCE_A", spec, subdim=False, uops_sha=shas)

    op = make({"v3": "?", "v4": "?"})
    for ver in ("v3", "v4"):
        try:
            op.compile(ver)
        except ValueError as e:
            import re
            m = re.search(r'uops_sha\["(v\d)"\]="([0-9a-f]+)"', str(e))
            assert m, str(e)
            shas = dict(op.uops_sha)
            shas[m.group(1)] = m.group(2)
            op = make(shas)
    op.compile("v3"); op.compile("v4")

    dve_ops.OPS.append(op)
    dve_ops.CUSTOM_DVE_SPECS[op.name] = op.spec
    return op


def _build_nc():
    import concourse.bacc as bacc
    import concourse.tile as tile
    import concourse.bass as bass
    import concourse.mybir as mybir

    rmr_op = _register_rmr()

    f32 = mybir.dt.float32
    AF = mybir.ActivationFunctionType
    OP = mybir.AluOpType
    MS = bass.MemorySpace

    nc = bacc.Bacc("TRN2", target_bir_lowering=False, debug=False)

    # packed inputs:
    #  WA [64, 31] = [A2_0 | A1s_0 | K | CB0 | la | pa0 | mla]
    #  RW [1, 21]  = [lb_row | b_row | psi0_row]
    d_WA = nc.dram_tensor("WA_in", [L, 31], f32, kind="ExternalInput").ap()
    d_RW = nc.dram_tensor("RW_in", [1, 21], f32, kind="ExternalInput").ap()
    d_P = nc.dram_tensor("P_out", [L, B], f32, kind="ExternalOutput").ap()

    with tile.TileContext(nc) as tc:
        with (
            tc.tile_pool(name="sb", bufs=1) as sb,
            tc.tile_pool(name="ps", bufs=2, space=MS.PSUM) as ps,
        ):
            def t(shape, tag):
                return sb.tile(shape, f32, tag=tag, name=tag)

            WA = t([L, 31], "WA")
            RW = t([1, 21], "RW")
            A2 = [WA[:, 0:7], t([L, B], "A2_1")]
            A1s = [WA[:, 7:14], t([L, B], "A1s_1")]
            K = WA[:, 14:21]
            CB0 = WA[:, 21:28]
            la = WA[:, 28:29]
            pa = [WA[:, 29:30], t([L, 1], "pa_1")]
            mla = WA[:, 30:31]                      # -la + ln|sigma|
            lb_row = RW[:, 0:7]
            b_row = RW[:, 7:14]
            psi = [RW[:, 14:21], t([1, B], "psi_1")]

            u = [t([L, 1], f"u{i}") for i in range(4)]
            r = [t([L, 1], f"r{i}") for i in range(4)]
            scr = [t([L, B], "scr0"), t([L, B], "scr1")]
            A2sw = t([L, B], "A2sw")
            paA1 = t([L, 1], "paA1")
            yf = t([L, 1], "yf")
            lnu = t([L, 1], "lnu")
            crow = t([1, B], "crow")
            yfr = t([1, B], "yfr")
            lncr = t([1, B], "lncr")
            psiT = t([1, B], "psiT")
            psiB = t([1, B], "psiB")
            PSBa = t([L, B], "PSBa")
            PSBb = t([L, B], "PSBb")
            T1a = t([L, B], "T1a")
            T1b = t([L, B], "T1b")
            qrow = t([1, B], "qrow")
            bq = t([1, B], "bq")
            BQ = t([L, B], "BQ")
            Pu = t([L, B], "Pu")
            Pf = t([L, B], "Pf")

            # ---- load inputs ----
            nc.sync.dma_start(out=WA[:], in_=d_WA)
            nc.gpsimd.dma_start(out=RW[:], in_=d_RW)

            # dummy Exp pulls the ACT table load into the head
            nc.scalar.activation(crow[:], lb_row, AF.Exp)

            def gps_ln(out_ap, yf_ap, x_ap):
                nc.gpsimd.tensor_copy(yf_ap, x_ap.bitcast(mybir.dt.uint32))
                nc.gpsimd.tensor_scalar(out=out_ap, in0=yf_ap, scalar1=GLC1,
                                        scalar2=GLC2, op0=OP.mult, op1=OP.add)

            epoch = 0
            fp = 0              # fold parity for pa/psi ping-pong
            switch_at = None
            deferred = {}
            prev_cb = None
            last_cb = None

            for it in range(2, ITERS + 1):
                par = it % 2
                par4 = it % 4
                if switch_at == it:
                    epoch = 1 - epoch
                    switch_at = None
                # RMR + recip
                in0 = CB0 if it == 2 else prev_cb[:]
                nc.vector._custom_dve(
                    rmr_op, out=scr[par][:], in0=in0, in1=A1s[epoch][:],
                    s0=BETA, s1=GAMMA, accum_out=r[par4][:],
                )
                nc.vector.reciprocal(u[par4][:], r[par4][:])
                # PE: CB_it (also emitted for it=200: needed for final q)
                cb = ps.tile([L, B], f32, tag="cb", bufs=3)
                ub = u[par4][:, 0:1].broadcast_to((L, L))
                rhs = A2sw if (switch_at == it + 1) else A2[epoch]
                nc.tensor.matmul(cb[:], ub, rhs[:], start=True, stop=True)
                prev_cb, last_cb = cb, cb

                # fold: snapshot u_it (Pool) and c_it (ACT copy of CB row).
                # c comes from a duplicate fold-only matmul so the loop's cb
                # tile keeps a single (DVE) reader - a second reader makes
                # Tile block the DVE sequencer on an EventSemaphore.
                if it in FOLD_ITS:
                    ne = 1 - epoch
                    nf = 1 - fp
                    cbf = ps.tile([L, B], f32, tag="cbf", bufs=1)
                    nc.tensor.matmul(cbf[:], ub, rhs[:], start=True, stop=True)
                    # ln u = -ln r (exact device recip), negated gpsimd-ln
                    nc.gpsimd.tensor_copy(yf[:], r[par4][:].bitcast(mybir.dt.uint32))
                    nc.gpsimd.tensor_scalar(out=lnu[:], in0=yf[:], scalar1=-GLC1,
                                            scalar2=-GLC2, op0=OP.mult, op1=OP.add)
                    nc.gpsimd.tensor_scalar(out=pa[nf][:], in0=lnu[:],
                                            scalar1=pa[fp][:], scalar2=la,
                                            op0=OP.add, op1=OP.add)
                    nc.gpsimd.tensor_scalar(out=paA1[:], in0=pa[nf][:],
                                            scalar1=mla, scalar2=None,
                                            op0=OP.add)
                    nc.scalar.activation(crow[:], cbf[0:1, :], AF.Copy)

                    def stage_b(nf=nf, fp=fp):
                        gps_ln(lncr[:], yfr[:], crow[:])
                        nc.gpsimd.tensor_tensor(out=psiT[:], in0=psi[fp][:],
                                                in1=lb_row, op=OP.add)
                        nc.gpsimd.tensor_tensor(out=psi[nf][:], in0=psiT[:],
                                                in1=lncr[:], op=OP.subtract)
                        nc.gpsimd.tensor_tensor(out=psiB[:], in0=psi[nf][:],
                                                in1=lb_row, op=OP.add)

                    def stage_c(nf=nf):
                        nc.gpsimd.partition_broadcast(PSBa[:], psi[nf][:])
                        nc.gpsimd.tensor_tensor(out=T1a[:], in0=K, in1=PSBa[:],
                                                op=OP.add)
                        nc.gpsimd.partition_broadcast(PSBb[:], psiB[:])
                        nc.gpsimd.tensor_tensor(out=T1b[:], in0=K, in1=PSBb[:],
                                                op=OP.add)

                    def stage_d(ne=ne, nf=nf, fp=fp):
                        nc.scalar.activation(A2[ne][:], T1a[:], AF.Exp,
                                             bias=pa[nf][:])
                        nc.scalar.activation(A2sw[:], T1a[:], AF.Exp,
                                             bias=pa[fp][:])
                        nc.scalar.activation(A1s[ne][:], T1b[:], AF.Exp,
                                             bias=paA1[:])

                    def stage_e(ne=ne):
                        nc.gpsimd.tensor_scalar(out=A1s[ne][:], in0=A1s[ne][:],
                                                scalar1=-1.0, scalar2=None,
                                                op0=OP.mult)

                    deferred.setdefault(it + 3, []).append(stage_b)
                    deferred.setdefault(it + 4, []).append(stage_c)
                    deferred.setdefault(it + 5, []).append(stage_d)
                    deferred.setdefault(it + 6, []).append(stage_e)
                    switch_at = it + DELAY
                    fp = nf

                for fn in deferred.pop(it, []):
                    fn()

            # ---- final: P = diag(u_200) A2 diag(b * (1/c_200)) ----
            fpar = ITERS % 4
            nc.gpsimd.tensor_scalar(out=Pu[:], in0=A2[epoch][:],
                                    scalar1=u[fpar][:], scalar2=None,
                                    op0=OP.mult)
            nc.vector.reciprocal(qrow[:], last_cb[0:1, :])
            nc.gpsimd.tensor_tensor(out=bq[:], in0=qrow[:], in1=b_row,
                                    op=OP.mult)
            nc.gpsimd.partition_broadcast(BQ[:], bq[:])
            nc.gpsimd.tensor_tensor(out=Pf[:], in0=Pu[:], in1=BQ[:],
                                    op=OP.mult)
            nc.sync.dma_start(out=d_P, in_=Pf[:])

    nc.compile()
    return nc


def _host_inputs(theta, phi, n, sens, err):
    f32 = np.float32
    theta = np.asarray(theta, f32); phi = np.asarray(phi, f32)
    n = np.asarray(n, f32); sens = np.asarray(sens, f32)
    err = np.asarray(err, f32)
    a = (n / n.sum()).astype(f32)
    e = np.exp((phi - phi.max()).astype(f32)); b = (e / e.sum()).astype(f32)
    C = ((n * sens)[:, None] * err[None, :]).astype(f32)
    K = ((theta - C) * f32(1.0 / EPS)).astype(f32)
    la = np.log(a).astype(f32)
    lb = np.log(b).astype(f32)
    lnsig = f32(np.log(-SIG))

    def lse(x, axis):
        m = x.max(axis=axis, keepdims=True)
        return (m + np.log(np.exp(x - m).sum(axis=axis, keepdims=True))
                ).squeeze(axis).astype(f32)

    def ftz(x):
        x = np.asarray(x, f32).copy()
        x[np.abs(x) < 1.17549435e-38] = 0.0
        return x

    # iteration 1 (log domain, max-stabilized LSE) on host
    f1 = (la - lse(K, 1)).astype(f32)
    g1 = (lb - lse(K + f1[:, None], 0)).astype(f32)
    pa0 = (f1 + la).astype(f32)
    psi0 = g1.astype(f32)
    A2_0 = ftz(np.exp((K + pa0[:, None] + psi0[None, :]).astype(f32)))
    A1s_0 = -ftz(np.exp((K + (pa0 - la + lnsig)[:, None]
                         + (psi0 + lb)[None, :]).astype(f32)))
    CB0 = np.broadcast_to(b[None, :], (L, B)).astype(f32)
    mla = (-la + lnsig).astype(f32)

    WA = np.concatenate(
        [A2_0, A1s_0, K, CB0,
         np.stack([la, pa0, mla], axis=1)], axis=1).astype(f32)
    RW = np.concatenate([lb, b, psi0])[None, :].astype(f32)
    return {"WA_in": np.ascontiguousarray(WA),
            "RW_in": np.ascontiguousarray(RW)}


def kernel(theta, phi, n, sens, err):
    if "nc" not in _CACHE:
        _CACHE["nc"] = _build_nc()
    nc = _CACHE["nc"]
    in_map = _host_inputs(theta, phi, n, sens, err)
    from concourse import bass_utils
    res = bass_utils.run_bass_kernel_spmd(nc, [in_map], [0])
    return np.asarray(res.results[0]["P_out"], dtype=np.float32)
